# revision 23
# baseline (speedup 1.0000x reference)
"""Trainium2 Bass kernel for nn_Attention_3GIN2 (GIN aggregation + per-head attention).

Reference computation (b=4, t=1024, dim=256, 8 heads of d=32):
    xh  = x reshaped to [b, h, t, d]
    agg = (1+eps)*xh + adj @ xh                    (GIN aggregation, per head)
    qkv = agg @ W_qkv ; q,k,v = split(qkv)
    attn = softmax(q*dim^-0.5 @ k.T)               (per head, returned as output!)
    out  = gelu((attn @ v) reshaped to [b, t, dim])

Sharding: 8 cores = 4 batches x 2 head-groups (4 heads each). Each core computes
its (b, 4-head) slice entirely on-chip and writes its 16MB attn chunk + out slab.

Device-side layout strategy:
  - adj.T (host-transposed) streams in; aggT[hd,t] = xh4.T @ adj.T + (1+eps)xT
    computed directly in "transposed" orientation so qk projections are natural.
    GIN/qk/scores stay full fp32 (attn_weight is a graded output).
  - qT/kT stored in 4 head-strips (partitions 32h) so consecutive score
    matmuls rotate PE row-groups and LDWEIGHTS pipelines with MATMULs.
  - scores computed in BOTH orientations ([t,s] fp32 for the softmax/attn
    output, [s,t] float32r for the attn@v contraction); softmax skips
    max-subtraction (scores bounded ~|25|, exp safely in f32 range), ACT
    accum_out provides attn row sums for free.
  - o path runs in float32r (PE 1-pass vs fp32's 4): exp(scoresT) emitted as
    f32r by ACT, v carries a ones column so the same matmul yields o-path
    row sums consistent with the f32r-rounded scores; o is PE-transposed
    back to [t,(h d)], scaled by 1/rowsum and gelu'd on ACT.
"""

import numpy as np

HEADS = 8
B = 4
T = 1024
DIM = 256
D = 32  # head dim
NH = 4  # heads per core
P = 128
NT = T // P  # 8 row tiles
SC = 512  # matmul free-dim chunk
SCALE = float(DIM) ** -0.5

_CACHE = {}

# scores matmuls in float32r (PE 1-pass, ~2.7x faster than fp32's 4-pass;
# rounds q/k to ~15 mantissa bits -> attn error ~1e-3-scale). False = full fp32.
SCORES_F32R = False


def _enable_ldw_opt():
    """Walrus is invoked with --enable-ldw-opt=false by default; consecutive
    matmuls sharing a stationary operand then reload weights every time.
    Rewrite the flag on the compiler command line (validated by the
    correctness check in test.py)."""
    import concourse.bass_utils as bu

    if getattr(bu.run_command, "_ldw_patched", False):
        return
    orig = bu.run_command

    def patched(argv, **kw):
        argv = [
            "--enable-ldw-opt=true" if a == "--enable-ldw-opt=false" else a
            for a in argv
        ]
        return orig(argv, **kw)

    patched._ldw_patched = True
    bu.run_command = patched


def _build():
    """Trace the per-core Bass program (identical on all 8 cores)."""
    import concourse.bass as bass
    import concourse.mybir as mybir
    import concourse.tile as tile
    from concourse import bacc
    from concourse.masks import make_identity

    _enable_ldw_opt()

    f32 = mybir.dt.float32
    f32r = mybir.dt.float32r
    EXP = mybir.ActivationFunctionType.Exp
    GELU = mybir.ActivationFunctionType.Gelu

    nc = bacc.Bacc("TRN2", target_bir_lowering=False, debug=False)

    adjT_d = nc.dram_tensor("adjT", (T, T), f32, kind="ExternalInput").ap()
    xs_d = nc.dram_tensor("xs", (T, P), f32, kind="ExternalInput").ap()
    xsT_d = nc.dram_tensor("xsT", (P, T), f32, kind="ExternalInput").ap()
    w4_d = nc.dram_tensor("w4", (P, 2 * D), f32, kind="ExternalInput").ap()
    wblk_d = nc.dram_tensor("wblk", (P, P), f32, kind="ExternalInput").ap()
    eps1_d = nc.dram_tensor("eps1", (P, 1), f32, kind="ExternalInput").ap()
    attn_d = nc.dram_tensor("attn_o", (NH, T, T), f32, kind="ExternalOutput").ap()
    out_d = nc.dram_tensor("out_o", (T, P), f32, kind="ExternalOutput").ap()

    with tile.TileContext(nc) as tc:
        with (
            tc.tile_pool(name="const", bufs=1) as constp,
            tc.tile_pool(name="mainp", bufs=1) as mainp,
        ):
            # ---- constants / small inputs
            ident = constp.tile([P, P], f32)
            make_identity(nc, ident)
            w4 = constp.tile([P, 2 * D], f32)
            nc.sync.dma_start(w4, w4_d)
            wblk = constp.tile([P, P], f32)
            nc.sync.dma_start(wblk, wblk_d)
            eps1 = constp.tile([P, 1], f32)
            nc.sync.dma_start(eps1, eps1_d)

            # ---- x (natural, for GIN lhsT) and (1+eps) * x.T
            xh4 = mainp.tile([P, NT, P], f32)  # [s_p, sn, (h d)]
            nc.sync.dma_start(xh4, xs_d.rearrange("(n p) c -> p n c", p=P))
            xsTs = mainp.tile([P, T], f32)  # [(h d), t]
            nc.sync.dma_start(xsTs, xsT_d)
            sxT = mainp.tile([P, T], f32)
            nc.vector.tensor_scalar_mul(sxT, xsTs, eps1[:, 0:1])

            aggT = mainp.tile([P, T], f32)  # [(h d), t]
            # q.T/k.T in 4 head-strips (partitions 32h..32h+32 = head h) so
            # consecutive score matmuls hit different PE row-groups and their
            # LDWEIGHTS overlap in-flight MATMULs.
            qT = mainp.tile([P, T], f32)  # [(h d), t], q pre-scaled
            kT = mainp.tile([P, T], f32)
            # v natural [t_p, tn, (h: v cols | ones col)] — the ones column makes
            # the oT matmul also produce sum_s exp(scoresT), giving o-path
            # denominators consistent with the fp32r-rounded scoresT.
            v5 = mainp.tile([P, NT, NH * (D + 1)], f32r)
            denT4 = mainp.tile([P, T], f32)  # o-path row sums at rows h*32
            recip2 = mainp.tile([P, NT * NH], f32)  # 1/denT, [t_p, (tn h)]
            denom = mainp.tile([P, NH * NT], f32)  # softmax row sums [t_p, (h tn)]
            recip = mainp.tile([P, NH * NT], f32)
            oT_sb = mainp.tile([P, T], f32)  # [(h d), t] unnormalized o.T
            ofin_v6 = mainp.tile([P, NT, P], f32)  # gelu(out) [t_p, tn, (h d)]

            # ---- GIN aggregation: aggT = xh4.T @ adjT + (1+eps)*x.T
            with (
                tc.tile_pool(name="adjp", bufs=1) as adjp,
                tc.tile_pool(name="spsum", bufs=2, space="PSUM") as spsum,
            ):
                adjT3 = adjp.tile([P, NT, T], f32)  # [s_p, sn, t]
                for c in range(NT):
                    nc.sync.dma_start(
                        adjT3[:, c : c + 1, :],
                        adjT_d[c * P : (c + 1) * P, :].rearrange(
                            "(n p) t -> p n t", p=P
                        ),
                    )
                # sn outer so the first matmuls start as soon as the first
                # adjT slab lands (two psum accumulators run in parallel)
                agg_ps = [spsum.tile([P, SC], f32, tag="agg", name=f"agg{t}") for t in range(2)]
                for sn in range(NT):
                    for tch in range(2):
                        nc.tensor.matmul(
                            agg_ps[tch],
                            lhsT=xh4[:, sn, :],
                            rhs=adjT3[:, sn, tch * SC : (tch + 1) * SC],
                            start=(sn == 0),
                            stop=(sn == NT - 1),
                        )
                for tch in range(2):
                    nc.vector.tensor_add(
                        aggT[:, tch * SC : (tch + 1) * SC],
                        agg_ps[tch],
                        sxT[:, tch * SC : (tch + 1) * SC],
                    )

                # ---- q/k projections: qkT = w4.T @ aggT (per head, K=32)
                for h in range(NH):
                    for tch in range(2):
                        qk_ps = spsum.tile([2 * D, SC], f32, tag="qk")
                        nc.tensor.matmul(
                            qk_ps,
                            lhsT=w4[h * D : (h + 1) * D, :],
                            rhs=aggT[h * D : (h + 1) * D, tch * SC : (tch + 1) * SC],
                            start=True,
                            stop=True,
                            tile_position=(h * D, 0),
                        )
                        nc.vector.tensor_copy(
                            qT[h * D : (h + 1) * D, tch * SC : (tch + 1) * SC],
                            qk_ps[0:D, :],
                        )
                        nc.vector.tensor_copy(
                            kT[h * D : (h + 1) * D, tch * SC : (tch + 1) * SC],
                            qk_ps[D : 2 * D, :],
                        )

                # ---- v (natural layout) via block-diagonal W_v, plus a ones
                # column per head for the o-path row sums
                nc.vector.memset(
                    v5.rearrange("p n (h c) -> p n h c", h=NH)[
                        :, :, :, D : D + 1
                    ].bitcast(mybir.dt.uint32),
                    0x3F800000,  # 1.0f
                )
                for tn in range(NT):
                    v_ps = spsum.tile([P, P], f32, tag="v")
                    nc.tensor.matmul(
                        v_ps,
                        lhsT=aggT[:, tn * P : (tn + 1) * P],
                        rhs=wblk,
                        start=True,
                        stop=True,
                    )
                    nc.vector.tensor_copy(
                        v5.rearrange("p n (h c) -> p n h c", h=NH)[:, tn, :, 0:D],
                        v_ps.rearrange("p (h c) -> p h c", h=NH),
                    )

            # fp32r copies of q/k for the o-path score matmuls (1-pass on PE)
            qTr = mainp.tile([P, T], f32r)
            nc.vector.tensor_copy(qTr, qT)
            kTr = mainp.tile([P, T], f32r)
            nc.vector.tensor_copy(kTr, kT)

            # ---- main attention loops (heads interleaved so consecutive
            # matmuls rotate PE row-groups and LDWEIGHTS overlaps)
            with (
                tc.tile_pool(name="mmps", bufs=3, space="PSUM") as mmps,
                tc.tile_pool(name="otps", bufs=2, space="PSUM") as otps,
                tc.tile_pool(name="epool", bufs=4) as epool,
                tc.tile_pool(name="attnp", bufs=6) as attnp,
                tc.tile_pool(name="etp", bufs=6) as etp,
                tc.tile_pool(name="onrmp", bufs=2) as onrmp,
            ):
                # attention-weights path: scores[t,s] -> exp -> normalize -> DMA
                a4s = {}
                for tn in range(NT):
                    for h in range(NH):
                        hs = slice(h * D, (h + 1) * D)
                        sc_ps = mmps.tile([P, T], f32, tag="sc", name=f"sc{h}_{tn}")
                        sq, sk = (qTr, kTr) if SCORES_F32R else (qT, kT)
                        for sch in range(2):
                            nc.tensor.matmul(
                                sc_ps[:, sch * SC : (sch + 1) * SC],
                                lhsT=sq[hs, tn * P : (tn + 1) * P],
                                rhs=sk[hs, sch * SC : (sch + 1) * SC],
                                start=True,
                                stop=True,
                                tile_position=(h * D, 0),
                            )
                        E = epool.tile([P, T], f32, tag="E", name=f"E{h}_{tn}")
                        idx = h * NT + tn
                        nc.scalar.activation(
                            E, sc_ps, EXP, accum_out=denom[:, idx : idx + 1]
                        )
                        nc.vector.reciprocal(
                            recip[:, idx : idx + 1], denom[:, idx : idx + 1]
                        )
                        if tn % 4 == 0:
                            a4s[h] = attnp.tile(
                                [P, 4, T], f32, tag="a4", name=f"a4_{h}_{tn // 4}"
                            )
                        nc.vector.tensor_scalar_mul(
                            a4s[h][:, tn % 4, :], E, recip[:, idx : idx + 1]
                        )
                        if tn % 4 == 3:
                            q4 = tn // 4
                            nc.sync.dma_start(
                                attn_d[h, q4 * 4 * P : (q4 + 1) * 4 * P, :].rearrange(
                                    "(n p) s -> p n s", p=P
                                ),
                                a4s[h],
                            )
                # o path: scoresT (fp32r) -> exp(fp32r) -> oT = v.T @ exp(scoresT)
                ET = etp.tile([P, NT, T], f32r, bufs=1)  # exp(scores.T) [s_p, sn, t]
                for h in range(NH):
                    hs = slice(h * D, (h + 1) * D)
                    for sn in range(NT):
                        scT_ps = mmps.tile([P, T], f32, tag="sc", name=f"scT{h}_{sn}")
                        for tch in range(2):
                            nc.tensor.matmul(
                                scT_ps[:, tch * SC : (tch + 1) * SC],
                                lhsT=kTr[hs, sn * P : (sn + 1) * P],
                                rhs=qTr[hs, tch * SC : (tch + 1) * SC],
                                start=True,
                                stop=True,
                                tile_position=(h * D, 0),
                            )
                        nc.scalar.activation(ET[:, sn, :], scT_ps, EXP)
                    for tch in range(2):
                        oT_ps = otps.tile(
                            [D + 1, SC], f32, tag="ot", name=f"ot{h}_{tch}"
                        )
                        for sn in range(NT):
                            nc.tensor.matmul(
                                oT_ps,
                                lhsT=v5[:, sn, h * (D + 1) : (h + 1) * (D + 1)],
                                rhs=ET[:, sn, tch * SC : (tch + 1) * SC],
                                start=(sn == 0),
                                stop=(sn == NT - 1),
                            )
                        nc.vector.tensor_copy(
                            oT_sb[hs, tch * SC : (tch + 1) * SC], oT_ps[0:D, :]
                        )
                        nc.vector.tensor_copy(
                            denT4[h * D : h * D + 1, tch * SC : (tch + 1) * SC],
                            oT_ps[D : D + 1, :],
                        )

                # ---- out = gelu(o * recip) back in natural layout
                for tn in range(NT):
                    on_ps = mmps.tile([P, P], f32, tag="sc", name=f"on{tn}")
                    nc.tensor.transpose(on_ps, oT_sb[:, tn * P : (tn + 1) * P], ident)
                    dn_ps = otps.tile([P, P], f32, tag="ot", name=f"dn{tn}")
                    nc.tensor.transpose(
                        dn_ps, denT4[:, tn * P : (tn + 1) * P], ident
                    )
                    nc.vector.reciprocal(
                        recip2[:, tn * NH : (tn + 1) * NH], dn_ps[:, 0 : NH * D : D]
                    )
                    onrm = onrmp.tile([P, NH, D], f32, tag="onrm")
                    rec4 = recip2[:, tn * NH : (tn + 1) * NH]  # [P, NH]
                    nc.vector.tensor_tensor(
                        onrm,
                        on_ps.rearrange("p (h d) -> p h d", h=NH),
                        rec4[:, :, None].to_broadcast([P, NH, D]),
                        mybir.AluOpType.mult,
                    )
                    nc.scalar.activation(
                        ofin_v6[:, tn, :], onrm.rearrange("p h d -> p (h d)"), GELU
                    )
                nc.sync.dma_start(out_d.rearrange("(n p) c -> p n c", p=P), ofin_v6)

    nc.compile()
    return nc


def _prep_inputs(x, adj, W_qkv, eps):
    """Host-side shard/layout prep: one input map per core."""
    eps1 = np.full((P, 1), 1.0 + float(np.asarray(eps).reshape(-1)[0]), np.float32)
    wq = np.ascontiguousarray(W_qkv[:, 0:D]) * np.float32(SCALE)
    wk = np.ascontiguousarray(W_qkv[:, D : 2 * D])
    wv = np.ascontiguousarray(W_qkv[:, 2 * D : 3 * D])
    w4 = np.zeros((P, 2 * D), np.float32)
    wblk = np.zeros((P, P), np.float32)
    for h in range(NH):
        w4[h * D : (h + 1) * D, 0:D] = wq
        w4[h * D : (h + 1) * D, D : 2 * D] = wk
        wblk[h * D : (h + 1) * D, h * D : (h + 1) * D] = wv

    in_maps = []
    for core in range(8):
        b, hg = core // 2, core % 2
        xs = np.ascontiguousarray(x[b, :, hg * P : (hg + 1) * P])
        in_maps.append(
            {
                "adjT": np.ascontiguousarray(adj[b].T),
                "xs": xs,
                "xsT": np.ascontiguousarray(xs.T),
                "w4": w4,
                "wblk": wblk,
                "eps1": eps1,
            }
        )
    return in_maps


def run(x, adj, W_qkv, eps, trace=False):
    """Run on 8 NeuronCores; returns (out, attn_weight, BassKernelResults)."""
    from concourse import bass_utils

    if "nc" not in _CACHE:
        _CACHE["nc"] = _build()
    nc = _CACHE["nc"]

    in_maps = _prep_inputs(
        np.asarray(x, np.float32), np.asarray(adj, np.float32),
        np.asarray(W_qkv, np.float32), np.asarray(eps, np.float32),
    )
    res = bass_utils.run_bass_kernel_spmd(
        nc, in_maps, core_ids=list(range(8)), trace=trace
    )

    out = np.empty((B, T, DIM), np.float32)
    attn = np.empty((B, HEADS, T, T), np.float32)
    for core in range(8):
        b, hg = core // 2, core % 2
        r = res.results[core]
        attn[b, hg * NH : (hg + 1) * NH] = r["attn_o"]
        out[b, :, hg * P : (hg + 1) * P] = r["out_o"]
    return out, attn, res


def kernel(x, adj, rep_adj_dis, W_qkv, eps):
    out, attn, _ = run(x, adj, W_qkv, eps, trace=False)
    return out, attn


# revision 24
# speedup vs baseline: 1.0031x; 1.0031x over previous
"""Trainium2 Bass kernel for nn_Attention_3GIN2 (GIN aggregation + per-head attention).

Reference computation (b=4, t=1024, dim=256, 8 heads of d=32):
    xh  = x reshaped to [b, h, t, d]
    agg = (1+eps)*xh + adj @ xh                    (GIN aggregation, per head)
    qkv = agg @ W_qkv ; q,k,v = split(qkv)
    attn = softmax(q*dim^-0.5 @ k.T)               (per head, returned as output!)
    out  = gelu((attn @ v) reshaped to [b, t, dim])

Sharding: 8 cores = 4 batches x 2 head-groups (4 heads each). Each core computes
its (b, 4-head) slice entirely on-chip and writes its 16MB attn chunk + out slab.

Device-side layout strategy:
  - adj.T (host-transposed) streams in; aggT[hd,t] = xh4.T @ adj.T + (1+eps)xT
    computed directly in "transposed" orientation so qk projections are natural.
    GIN/qk/scores stay full fp32 (attn_weight is a graded output).
  - qT/kT stored in 4 head-strips (partitions 32h) so consecutive score
    matmuls rotate PE row-groups and LDWEIGHTS pipelines with MATMULs.
  - scores computed in BOTH orientations ([t,s] fp32 for the softmax/attn
    output, [s,t] float32r for the attn@v contraction); softmax skips
    max-subtraction (scores bounded ~|25|, exp safely in f32 range), ACT
    accum_out provides attn row sums for free.
  - o path runs in float32r (PE 1-pass vs fp32's 4): exp(scoresT) emitted as
    f32r by ACT, v carries a ones column so the same matmul yields o-path
    row sums consistent with the f32r-rounded scores; o is PE-transposed
    back to [t,(h d)], scaled by 1/rowsum and gelu'd on ACT.
"""

import numpy as np

HEADS = 8
B = 4
T = 1024
DIM = 256
D = 32  # head dim
NH = 4  # heads per core
P = 128
NT = T // P  # 8 row tiles
SC = 512  # matmul free-dim chunk
SCALE = float(DIM) ** -0.5

_CACHE = {}

# scores matmuls in float32r (PE 1-pass, ~2.7x faster than fp32's 4-pass;
# rounds q/k to ~15 mantissa bits -> attn error ~1e-3-scale). False = full fp32.
SCORES_F32R = False


def _build():
    """Trace the per-core Bass program (identical on all 8 cores)."""
    import concourse.bass as bass
    import concourse.mybir as mybir
    import concourse.tile as tile
    from concourse import bacc
    from concourse.masks import make_identity

    f32 = mybir.dt.float32
    f32r = mybir.dt.float32r
    EXP = mybir.ActivationFunctionType.Exp
    GELU = mybir.ActivationFunctionType.Gelu

    nc = bacc.Bacc("TRN2", target_bir_lowering=False, debug=False)

    adjT_d = nc.dram_tensor("adjT", (T, T), f32, kind="ExternalInput").ap()
    xs_d = nc.dram_tensor("xs", (T, P), f32, kind="ExternalInput").ap()
    xsT_d = nc.dram_tensor("xsT", (P, T), f32, kind="ExternalInput").ap()
    w4_d = nc.dram_tensor("w4", (P, 2 * D), f32, kind="ExternalInput").ap()
    wblk_d = nc.dram_tensor("wblk", (P, P), f32, kind="ExternalInput").ap()
    eps1_d = nc.dram_tensor("eps1", (P, 1), f32, kind="ExternalInput").ap()
    attn_d = nc.dram_tensor("attn_o", (NH, T, T), f32, kind="ExternalOutput").ap()
    out_d = nc.dram_tensor("out_o", (T, P), f32, kind="ExternalOutput").ap()

    with tile.TileContext(nc) as tc:
        with (
            tc.tile_pool(name="const", bufs=1) as constp,
            tc.tile_pool(name="mainp", bufs=1) as mainp,
        ):
            # ---- constants / small inputs
            ident = constp.tile([P, P], f32)
            make_identity(nc, ident)
            w4 = constp.tile([P, 2 * D], f32)
            nc.sync.dma_start(w4, w4_d)
            wblk = constp.tile([P, P], f32)
            nc.sync.dma_start(wblk, wblk_d)
            eps1 = constp.tile([P, 1], f32)
            nc.sync.dma_start(eps1, eps1_d)

            # ---- x (natural, for GIN lhsT) and (1+eps) * x.T
            xh4 = mainp.tile([P, NT, P], f32)  # [s_p, sn, (h d)]
            nc.sync.dma_start(xh4, xs_d.rearrange("(n p) c -> p n c", p=P))
            xsTs = mainp.tile([P, T], f32)  # [(h d), t]
            nc.sync.dma_start(xsTs, xsT_d)
            sxT = mainp.tile([P, T], f32)
            nc.vector.tensor_scalar_mul(sxT, xsTs, eps1[:, 0:1])

            aggT = mainp.tile([P, T], f32)  # [(h d), t]
            # q.T/k.T in 4 head-strips (partitions 32h..32h+32 = head h) so
            # consecutive score matmuls hit different PE row-groups and their
            # LDWEIGHTS overlap in-flight MATMULs.
            qT = mainp.tile([P, T], f32)  # [(h d), t], q pre-scaled
            kT = mainp.tile([P, T], f32)
            # v natural [t_p, tn, (h: v cols | ones col)] — the ones column makes
            # the oT matmul also produce sum_s exp(scoresT), giving o-path
            # denominators consistent with the fp32r-rounded scoresT.
            v5 = mainp.tile([P, NT, NH * (D + 1)], f32r)
            denT4 = mainp.tile([P, T], f32)  # o-path row sums at rows h*32
            recip2 = mainp.tile([P, NT * NH], f32)  # 1/denT, [t_p, (tn h)]
            denom = mainp.tile([P, NH * NT], f32)  # softmax row sums [t_p, (h tn)]
            recip = mainp.tile([P, NH * NT], f32)
            oT_sb = mainp.tile([P, T], f32)  # [(h d), t] unnormalized o.T
            ofin_v6 = mainp.tile([P, NT, P], f32)  # gelu(out) [t_p, tn, (h d)]

            # ---- GIN aggregation: aggT = xh4.T @ adjT + (1+eps)*x.T
            with (
                tc.tile_pool(name="adjp", bufs=1) as adjp,
                tc.tile_pool(name="spsum", bufs=2, space="PSUM") as spsum,
            ):
                adjT3 = adjp.tile([P, NT, T], f32)  # [s_p, sn, t]
                for c in range(NT):
                    nc.sync.dma_start(
                        adjT3[:, c : c + 1, :],
                        adjT_d[c * P : (c + 1) * P, :].rearrange(
                            "(n p) t -> p n t", p=P
                        ),
                    )
                # sn outer so the first matmuls start as soon as the first
                # adjT slab lands (two psum accumulators run in parallel)
                agg_ps = [spsum.tile([P, SC], f32, tag="agg", name=f"agg{t}") for t in range(2)]
                for sn in range(NT):
                    for tch in range(2):
                        nc.tensor.matmul(
                            agg_ps[tch],
                            lhsT=xh4[:, sn, :],
                            rhs=adjT3[:, sn, tch * SC : (tch + 1) * SC],
                            start=(sn == 0),
                            stop=(sn == NT - 1),
                        )
                for tch in range(2):
                    nc.vector.tensor_add(
                        aggT[:, tch * SC : (tch + 1) * SC],
                        agg_ps[tch],
                        sxT[:, tch * SC : (tch + 1) * SC],
                    )

                # ---- q/k projections: qkT = w4.T @ aggT (per head, K=32)
                for h in range(NH):
                    for tch in range(2):
                        qk_ps = spsum.tile([2 * D, SC], f32, tag="qk")
                        nc.tensor.matmul(
                            qk_ps,
                            lhsT=w4[h * D : (h + 1) * D, :],
                            rhs=aggT[h * D : (h + 1) * D, tch * SC : (tch + 1) * SC],
                            start=True,
                            stop=True,
                            tile_position=(h * D, 0),
                        )
                        nc.vector.tensor_copy(
                            qT[h * D : (h + 1) * D, tch * SC : (tch + 1) * SC],
                            qk_ps[0:D, :],
                        )
                        nc.vector.tensor_copy(
                            kT[h * D : (h + 1) * D, tch * SC : (tch + 1) * SC],
                            qk_ps[D : 2 * D, :],
                        )

                # ---- v (natural layout) via block-diagonal W_v, plus a ones
                # column per head for the o-path row sums
                nc.vector.memset(
                    v5.rearrange("p n (h c) -> p n h c", h=NH)[
                        :, :, :, D : D + 1
                    ].bitcast(mybir.dt.uint32),
                    0x3F800000,  # 1.0f
                )
                for tn in range(NT):
                    v_ps = spsum.tile([P, P], f32, tag="v")
                    nc.tensor.matmul(
                        v_ps,
                        lhsT=aggT[:, tn * P : (tn + 1) * P],
                        rhs=wblk,
                        start=True,
                        stop=True,
                    )
                    nc.vector.tensor_copy(
                        v5.rearrange("p n (h c) -> p n h c", h=NH)[:, tn, :, 0:D],
                        v_ps.rearrange("p (h c) -> p h c", h=NH),
                    )

            # fp32r copies of q/k for the o-path score matmuls (1-pass on PE)
            qTr = mainp.tile([P, T], f32r)
            nc.vector.tensor_copy(qTr, qT)
            kTr = mainp.tile([P, T], f32r)
            nc.vector.tensor_copy(kTr, kT)

            # ---- main attention loops (heads interleaved so consecutive
            # matmuls rotate PE row-groups and LDWEIGHTS overlaps)
            with (
                tc.tile_pool(name="mmps", bufs=3, space="PSUM") as mmps,
                tc.tile_pool(name="otps", bufs=2, space="PSUM") as otps,
                tc.tile_pool(name="epool", bufs=4) as epool,
                tc.tile_pool(name="attnp", bufs=6) as attnp,
                tc.tile_pool(name="etp", bufs=6) as etp,
                tc.tile_pool(name="onrmp", bufs=2) as onrmp,
            ):
                # attention-weights path: scores[t,s] -> exp -> normalize -> DMA
                a4s = {}
                for tn in range(NT):
                    for h in range(NH):
                        hs = slice(h * D, (h + 1) * D)
                        sc_ps = mmps.tile([P, T], f32, tag="sc", name=f"sc{h}_{tn}")
                        sq, sk = (qTr, kTr) if SCORES_F32R else (qT, kT)
                        for sch in range(2):
                            nc.tensor.matmul(
                                sc_ps[:, sch * SC : (sch + 1) * SC],
                                lhsT=sq[hs, tn * P : (tn + 1) * P],
                                rhs=sk[hs, sch * SC : (sch + 1) * SC],
                                start=True,
                                stop=True,
                                tile_position=(h * D, 0),
                            )
                        E = epool.tile([P, T], f32, tag="E", name=f"E{h}_{tn}")
                        idx = h * NT + tn
                        nc.scalar.activation(
                            E, sc_ps, EXP, accum_out=denom[:, idx : idx + 1]
                        )
                        nc.vector.reciprocal(
                            recip[:, idx : idx + 1], denom[:, idx : idx + 1]
                        )
                        if tn % 4 == 0:
                            a4s[h] = attnp.tile(
                                [P, 4, T], f32, tag="a4", name=f"a4_{h}_{tn // 4}"
                            )
                        nc.vector.tensor_scalar_mul(
                            a4s[h][:, tn % 4, :], E, recip[:, idx : idx + 1]
                        )
                        if tn % 4 == 3:
                            q4 = tn // 4
                            nc.sync.dma_start(
                                attn_d[h, q4 * 4 * P : (q4 + 1) * 4 * P, :].rearrange(
                                    "(n p) s -> p n s", p=P
                                ),
                                a4s[h],
                            )
                # o path: scoresT (fp32r) -> exp(fp32r) -> oT = v.T @ exp(scoresT)
                ET = etp.tile([P, NT, T], f32r, bufs=1)  # exp(scores.T) [s_p, sn, t]
                for h in range(NH):
                    hs = slice(h * D, (h + 1) * D)
                    for sn in range(NT):
                        scT_ps = mmps.tile([P, T], f32, tag="sc", name=f"scT{h}_{sn}")
                        for tch in range(2):
                            nc.tensor.matmul(
                                scT_ps[:, tch * SC : (tch + 1) * SC],
                                lhsT=kTr[hs, sn * P : (sn + 1) * P],
                                rhs=qTr[hs, tch * SC : (tch + 1) * SC],
                                start=True,
                                stop=True,
                                tile_position=(h * D, 0),
                            )
                        nc.scalar.activation(ET[:, sn, :], scT_ps, EXP)
                    for tch in range(2):
                        oT_ps = otps.tile(
                            [D + 1, SC], f32, tag="ot", name=f"ot{h}_{tch}"
                        )
                        for sn in range(NT):
                            nc.tensor.matmul(
                                oT_ps,
                                lhsT=v5[:, sn, h * (D + 1) : (h + 1) * (D + 1)],
                                rhs=ET[:, sn, tch * SC : (tch + 1) * SC],
                                start=(sn == 0),
                                stop=(sn == NT - 1),
                            )
                        nc.vector.tensor_copy(
                            oT_sb[hs, tch * SC : (tch + 1) * SC], oT_ps[0:D, :]
                        )
                        nc.vector.tensor_copy(
                            denT4[h * D : h * D + 1, tch * SC : (tch + 1) * SC],
                            oT_ps[D : D + 1, :],
                        )

                # ---- out = gelu(o * recip) back in natural layout
                for tn in range(NT):
                    on_ps = mmps.tile([P, P], f32, tag="sc", name=f"on{tn}")
                    nc.tensor.transpose(on_ps, oT_sb[:, tn * P : (tn + 1) * P], ident)
                    dn_ps = otps.tile([P, P], f32, tag="ot", name=f"dn{tn}")
                    nc.tensor.transpose(
                        dn_ps, denT4[:, tn * P : (tn + 1) * P], ident
                    )
                    nc.vector.reciprocal(
                        recip2[:, tn * NH : (tn + 1) * NH], dn_ps[:, 0 : NH * D : D]
                    )
                    onrm = onrmp.tile([P, NH, D], f32, tag="onrm")
                    rec4 = recip2[:, tn * NH : (tn + 1) * NH]  # [P, NH]
                    nc.vector.tensor_tensor(
                        onrm,
                        on_ps.rearrange("p (h d) -> p h d", h=NH),
                        rec4[:, :, None].to_broadcast([P, NH, D]),
                        mybir.AluOpType.mult,
                    )
                    nc.scalar.activation(
                        ofin_v6[:, tn, :], onrm.rearrange("p h d -> p (h d)"), GELU
                    )
                nc.sync.dma_start(out_d.rearrange("(n p) c -> p n c", p=P), ofin_v6)

    nc.compile()
    return nc


def _prep_inputs(x, adj, W_qkv, eps):
    """Host-side shard/layout prep: one input map per core."""
    eps1 = np.full((P, 1), 1.0 + float(np.asarray(eps).reshape(-1)[0]), np.float32)
    wq = np.ascontiguousarray(W_qkv[:, 0:D]) * np.float32(SCALE)
    wk = np.ascontiguousarray(W_qkv[:, D : 2 * D])
    wv = np.ascontiguousarray(W_qkv[:, 2 * D : 3 * D])
    w4 = np.zeros((P, 2 * D), np.float32)
    wblk = np.zeros((P, P), np.float32)
    for h in range(NH):
        w4[h * D : (h + 1) * D, 0:D] = wq
        w4[h * D : (h + 1) * D, D : 2 * D] = wk
        wblk[h * D : (h + 1) * D, h * D : (h + 1) * D] = wv

    in_maps = []
    for core in range(8):
        b, hg = core // 2, core % 2
        xs = np.ascontiguousarray(x[b, :, hg * P : (hg + 1) * P])
        in_maps.append(
            {
                "adjT": np.ascontiguousarray(adj[b].T),
                "xs": xs,
                "xsT": np.ascontiguousarray(xs.T),
                "w4": w4,
                "wblk": wblk,
                "eps1": eps1,
            }
        )
    return in_maps


def run(x, adj, W_qkv, eps, trace=False):
    """Run on 8 NeuronCores; returns (out, attn_weight, BassKernelResults)."""
    from concourse import bass_utils

    if "nc" not in _CACHE:
        _CACHE["nc"] = _build()
    nc = _CACHE["nc"]

    in_maps = _prep_inputs(
        np.asarray(x, np.float32), np.asarray(adj, np.float32),
        np.asarray(W_qkv, np.float32), np.asarray(eps, np.float32),
    )
    res = bass_utils.run_bass_kernel_spmd(
        nc, in_maps, core_ids=list(range(8)), trace=trace
    )

    out = np.empty((B, T, DIM), np.float32)
    attn = np.empty((B, HEADS, T, T), np.float32)
    for core in range(8):
        b, hg = core // 2, core % 2
        r = res.results[core]
        attn[b, hg * NH : (hg + 1) * NH] = r["attn_o"]
        out[b, :, hg * P : (hg + 1) * P] = r["out_o"]
    return out, attn, res


def kernel(x, adj, rep_adj_dis, W_qkv, eps):
    out, attn, _ = run(x, adj, W_qkv, eps, trace=False)
    return out, attn


# revision 26
# speedup vs baseline: 1.0156x; 1.0125x over previous
"""Trainium2 Bass kernel for nn_Attention_3GIN2 (GIN aggregation + per-head attention).

Reference computation (b=4, t=1024, dim=256, 8 heads of d=32):
    xh  = x reshaped to [b, h, t, d]
    agg = (1+eps)*xh + adj @ xh                    (GIN aggregation, per head)
    qkv = agg @ W_qkv ; q,k,v = split(qkv)
    attn = softmax(q*dim^-0.5 @ k.T)               (per head, returned as output!)
    out  = gelu((attn @ v) reshaped to [b, t, dim])

Sharding: 8 cores = 4 batches x 2 head-groups (4 heads each). Each core computes
its (b, 4-head) slice entirely on-chip and writes its 16MB attn chunk + out slab.

Device-side layout strategy:
  - adj.T (host-transposed) streams in; aggT[hd,t] = xh4.T @ adj.T + (1+eps)xT
    computed directly in "transposed" orientation so qk projections are natural.
    GIN/qk/scores stay full fp32 (attn_weight is a graded output).
  - qT/kT stored in 4 head-strips (partitions 32h) so consecutive score
    matmuls rotate PE row-groups and LDWEIGHTS pipelines with MATMULs.
  - scores computed in BOTH orientations ([t,s] fp32 for the softmax/attn
    output, [s,t] float32r for the attn@v contraction); softmax skips
    max-subtraction (scores bounded ~|25|, exp safely in f32 range), ACT
    accum_out provides attn row sums for free.
  - o path runs in float32r (PE 1-pass vs fp32's 4): exp(scoresT) emitted as
    f32r by ACT, v carries a ones column so the same matmul yields o-path
    row sums consistent with the f32r-rounded scores; o is PE-transposed
    back to [t,(h d)], scaled by 1/rowsum and gelu'd on ACT.
"""

import numpy as np

HEADS = 8
B = 4
T = 1024
DIM = 256
D = 32  # head dim
NH = 4  # heads per core
P = 128
NT = T // P  # 8 row tiles
SC = 512  # matmul free-dim chunk
SCALE = float(DIM) ** -0.5

_CACHE = {}

# scores matmuls in float32r (PE 1-pass, ~2.7x faster than fp32's 4-pass;
# rounds q/k to ~15 mantissa bits -> attn error ~1e-3-scale). False = full fp32.
SCORES_F32R = False


def _build():
    """Trace the per-core Bass program (identical on all 8 cores)."""
    import concourse.bass as bass
    import concourse.mybir as mybir
    import concourse.tile as tile
    from concourse import bacc
    from concourse.masks import make_identity

    f32 = mybir.dt.float32
    f32r = mybir.dt.float32r
    EXP = mybir.ActivationFunctionType.Exp
    GELU = mybir.ActivationFunctionType.Gelu

    nc = bacc.Bacc("TRN2", target_bir_lowering=False, debug=False)

    adjT_d = nc.dram_tensor("adjT", (T, T), f32, kind="ExternalInput").ap()
    xs_d = nc.dram_tensor("xs", (T, P), f32, kind="ExternalInput").ap()
    xsT_d = nc.dram_tensor("xsT", (P, T), f32, kind="ExternalInput").ap()
    w4_d = nc.dram_tensor("w4", (P, 2 * D), f32, kind="ExternalInput").ap()
    wblk_d = nc.dram_tensor("wblk", (P, P), f32, kind="ExternalInput").ap()
    eps1_d = nc.dram_tensor("eps1", (P, 1), f32, kind="ExternalInput").ap()
    attn_d = nc.dram_tensor("attn_o", (NH, T, T), f32, kind="ExternalOutput").ap()
    out_d = nc.dram_tensor("out_o", (T, P), f32, kind="ExternalOutput").ap()

    with tile.TileContext(nc) as tc:
        with (
            tc.tile_pool(name="const", bufs=1) as constp,
            tc.tile_pool(name="mainp", bufs=1) as mainp,
        ):
            # ---- x (natural, for GIN lhsT) first: the GIN matmuls need only
            # xh4 + the first adjT chunk, so everything else queues after the
            # critical-path DMAs (saves ~10us of PE-idle head).
            xh4 = mainp.tile([P, NT, P], f32)  # [s_p, sn, (h d)]
            nc.sync.dma_start(xh4, xs_d.rearrange("(n p) c -> p n c", p=P))

            ident = constp.tile([P, P], f32)
            make_identity(nc, ident)

            aggT = mainp.tile([P, T], f32)  # [(h d), t]
            # q.T/k.T in 4 head-strips (partitions 32h..32h+32 = head h) so
            # consecutive score matmuls hit different PE row-groups and their
            # LDWEIGHTS overlap in-flight MATMULs.
            qT = mainp.tile([P, T], f32)  # [(h d), t], q pre-scaled
            kT = mainp.tile([P, T], f32)
            # v natural [t_p, tn, (h: v cols | ones col)] — the ones column makes
            # the oT matmul also produce sum_s exp(scoresT), giving o-path
            # denominators consistent with the fp32r-rounded scoresT.
            v5 = mainp.tile([P, NT, NH * (D + 1)], f32r)
            denT4 = mainp.tile([P, T], f32)  # o-path row sums at rows h*32
            recip2 = mainp.tile([P, NT * NH], f32)  # 1/denT, [t_p, (tn h)]
            denom = mainp.tile([P, NH * NT], f32)  # softmax row sums [t_p, (h tn)]
            recip = mainp.tile([P, NH * NT], f32)
            oT_sb = mainp.tile([P, T], f32)  # [(h d), t] unnormalized o.T
            ofin_v6 = mainp.tile([P, NT, P], f32)  # gelu(out) [t_p, tn, (h d)]

            # ---- GIN aggregation: aggT = xh4.T @ adjT + (1+eps)*x.T
            with (
                tc.tile_pool(name="adjp", bufs=1) as adjp,
                tc.tile_pool(name="spsum", bufs=2, space="PSUM") as spsum,
            ):
                adjT3 = adjp.tile([P, NT, T], f32)  # [s_p, sn, t]
                for c in range(4):
                    nc.sync.dma_start(
                        adjT3[:, 2 * c : 2 * c + 2, :],
                        adjT_d[2 * c * P : (2 * c + 2) * P, :].rearrange(
                            "(n p) t -> p n t", p=P
                        ),
                    )
                # deferred non-critical inputs
                xsTs = mainp.tile([P, T], f32)  # [(h d), t]
                nc.sync.dma_start(xsTs, xsT_d)
                w4 = constp.tile([P, 2 * D], f32)
                nc.sync.dma_start(w4, w4_d)
                wblk = constp.tile([P, P], f32)
                nc.sync.dma_start(wblk, wblk_d)
                eps1 = constp.tile([P, 1], f32)
                nc.sync.dma_start(eps1, eps1_d)
                sxT = mainp.tile([P, T], f32)
                nc.vector.tensor_scalar_mul(sxT, xsTs, eps1[:, 0:1])
                # sn outer so the first matmuls start as soon as the first
                # adjT slab lands (two psum accumulators run in parallel)
                agg_ps = [spsum.tile([P, SC], f32, tag="agg", name=f"agg{t}") for t in range(2)]
                for sn in range(NT):
                    for tch in range(2):
                        nc.tensor.matmul(
                            agg_ps[tch],
                            lhsT=xh4[:, sn, :],
                            rhs=adjT3[:, sn, tch * SC : (tch + 1) * SC],
                            start=(sn == 0),
                            stop=(sn == NT - 1),
                        )
                for tch in range(2):
                    nc.vector.tensor_add(
                        aggT[:, tch * SC : (tch + 1) * SC],
                        agg_ps[tch],
                        sxT[:, tch * SC : (tch + 1) * SC],
                    )

                # ---- q/k projections: qkT = w4.T @ aggT (per head, K=32)
                for h in range(NH):
                    for tch in range(2):
                        qk_ps = spsum.tile([2 * D, SC], f32, tag="qk")
                        nc.tensor.matmul(
                            qk_ps,
                            lhsT=w4[h * D : (h + 1) * D, :],
                            rhs=aggT[h * D : (h + 1) * D, tch * SC : (tch + 1) * SC],
                            start=True,
                            stop=True,
                            tile_position=(h * D, 0),
                        )
                        nc.vector.tensor_copy(
                            qT[h * D : (h + 1) * D, tch * SC : (tch + 1) * SC],
                            qk_ps[0:D, :],
                        )
                        nc.vector.tensor_copy(
                            kT[h * D : (h + 1) * D, tch * SC : (tch + 1) * SC],
                            qk_ps[D : 2 * D, :],
                        )

                # ---- v (natural layout) via block-diagonal W_v, plus a ones
                # column per head for the o-path row sums
                nc.vector.memset(
                    v5.rearrange("p n (h c) -> p n h c", h=NH)[
                        :, :, :, D : D + 1
                    ].bitcast(mybir.dt.uint32),
                    0x3F800000,  # 1.0f
                )
                for tn in range(NT):
                    v_ps = spsum.tile([P, P], f32, tag="v")
                    nc.tensor.matmul(
                        v_ps,
                        lhsT=aggT[:, tn * P : (tn + 1) * P],
                        rhs=wblk,
                        start=True,
                        stop=True,
                    )
                    nc.vector.tensor_copy(
                        v5.rearrange("p n (h c) -> p n h c", h=NH)[:, tn, :, 0:D],
                        v_ps.rearrange("p (h c) -> p h c", h=NH),
                    )

            # fp32r copies of q/k for the o-path score matmuls (1-pass on PE)
            qTr = mainp.tile([P, T], f32r)
            nc.vector.tensor_copy(qTr, qT)
            kTr = mainp.tile([P, T], f32r)
            nc.vector.tensor_copy(kTr, kT)

            # ---- main attention loops (heads interleaved so consecutive
            # matmuls rotate PE row-groups and LDWEIGHTS overlaps)
            with (
                tc.tile_pool(name="mmps", bufs=3, space="PSUM") as mmps,
                tc.tile_pool(name="otps", bufs=2, space="PSUM") as otps,
                tc.tile_pool(name="epool", bufs=4) as epool,
                tc.tile_pool(name="attnp", bufs=6) as attnp,
                tc.tile_pool(name="etp", bufs=6) as etp,
                tc.tile_pool(name="onrmp", bufs=2) as onrmp,
            ):
                # attention-weights path: scores[t,s] -> exp -> normalize -> DMA
                a4s = {}
                for tn in range(NT):
                    for h in range(NH):
                        hs = slice(h * D, (h + 1) * D)
                        sc_ps = mmps.tile([P, T], f32, tag="sc", name=f"sc{h}_{tn}")
                        sq, sk = (qTr, kTr) if SCORES_F32R else (qT, kT)
                        for sch in range(2):
                            nc.tensor.matmul(
                                sc_ps[:, sch * SC : (sch + 1) * SC],
                                lhsT=sq[hs, tn * P : (tn + 1) * P],
                                rhs=sk[hs, sch * SC : (sch + 1) * SC],
                                start=True,
                                stop=True,
                                tile_position=(h * D, 0),
                            )
                        E = epool.tile([P, T], f32, tag="E", name=f"E{h}_{tn}")
                        idx = h * NT + tn
                        nc.scalar.activation(
                            E, sc_ps, EXP, accum_out=denom[:, idx : idx + 1]
                        )
                        nc.vector.reciprocal(
                            recip[:, idx : idx + 1], denom[:, idx : idx + 1]
                        )
                        if tn % 4 == 0:
                            a4s[h] = attnp.tile(
                                [P, 4, T], f32, tag="a4", name=f"a4_{h}_{tn // 4}"
                            )
                        nc.vector.tensor_scalar_mul(
                            a4s[h][:, tn % 4, :], E, recip[:, idx : idx + 1]
                        )
                        if tn % 4 == 3:
                            q4 = tn // 4
                            nc.sync.dma_start(
                                attn_d[h, q4 * 4 * P : (q4 + 1) * 4 * P, :].rearrange(
                                    "(n p) s -> p n s", p=P
                                ),
                                a4s[h],
                            )
                # o path: scoresT (fp32r) -> exp(fp32r) -> oT = v.T @ exp(scoresT)
                ET = etp.tile([P, NT, T], f32r, bufs=1)  # exp(scores.T) [s_p, sn, t]
                for h in range(NH):
                    hs = slice(h * D, (h + 1) * D)
                    for sn in range(NT):
                        scT_ps = mmps.tile([P, T], f32, tag="sc", name=f"scT{h}_{sn}")
                        for tch in range(2):
                            nc.tensor.matmul(
                                scT_ps[:, tch * SC : (tch + 1) * SC],
                                lhsT=kTr[hs, sn * P : (sn + 1) * P],
                                rhs=qTr[hs, tch * SC : (tch + 1) * SC],
                                start=True,
                                stop=True,
                                tile_position=(h * D, 0),
                            )
                        nc.scalar.activation(ET[:, sn, :], scT_ps, EXP)
                    for tch in range(2):
                        oT_ps = otps.tile(
                            [D + 1, SC], f32, tag="ot", name=f"ot{h}_{tch}"
                        )
                        for sn in range(NT):
                            nc.tensor.matmul(
                                oT_ps,
                                lhsT=v5[:, sn, h * (D + 1) : (h + 1) * (D + 1)],
                                rhs=ET[:, sn, tch * SC : (tch + 1) * SC],
                                start=(sn == 0),
                                stop=(sn == NT - 1),
                            )
                        nc.vector.tensor_copy(
                            oT_sb[hs, tch * SC : (tch + 1) * SC], oT_ps[0:D, :]
                        )
                        nc.vector.tensor_copy(
                            denT4[h * D : h * D + 1, tch * SC : (tch + 1) * SC],
                            oT_ps[D : D + 1, :],
                        )

                # ---- out = gelu(o * recip) back in natural layout
                for tn in range(NT):
                    on_ps = mmps.tile([P, P], f32, tag="sc", name=f"on{tn}")
                    nc.tensor.transpose(on_ps, oT_sb[:, tn * P : (tn + 1) * P], ident)
                    dn_ps = otps.tile([P, P], f32, tag="ot", name=f"dn{tn}")
                    nc.tensor.transpose(
                        dn_ps, denT4[:, tn * P : (tn + 1) * P], ident
                    )
                    nc.vector.reciprocal(
                        recip2[:, tn * NH : (tn + 1) * NH], dn_ps[:, 0 : NH * D : D]
                    )
                    onrm = onrmp.tile([P, NH, D], f32, tag="onrm")
                    rec4 = recip2[:, tn * NH : (tn + 1) * NH]  # [P, NH]
                    nc.vector.tensor_tensor(
                        onrm,
                        on_ps.rearrange("p (h d) -> p h d", h=NH),
                        rec4[:, :, None].to_broadcast([P, NH, D]),
                        mybir.AluOpType.mult,
                    )
                    nc.scalar.activation(
                        ofin_v6[:, tn, :], onrm.rearrange("p h d -> p (h d)"), GELU
                    )
                nc.sync.dma_start(out_d.rearrange("(n p) c -> p n c", p=P), ofin_v6)

    nc.compile()
    return nc


def _prep_inputs(x, adj, W_qkv, eps):
    """Host-side shard/layout prep: one input map per core."""
    eps1 = np.full((P, 1), 1.0 + float(np.asarray(eps).reshape(-1)[0]), np.float32)
    wq = np.ascontiguousarray(W_qkv[:, 0:D]) * np.float32(SCALE)
    wk = np.ascontiguousarray(W_qkv[:, D : 2 * D])
    wv = np.ascontiguousarray(W_qkv[:, 2 * D : 3 * D])
    w4 = np.zeros((P, 2 * D), np.float32)
    wblk = np.zeros((P, P), np.float32)
    for h in range(NH):
        w4[h * D : (h + 1) * D, 0:D] = wq
        w4[h * D : (h + 1) * D, D : 2 * D] = wk
        wblk[h * D : (h + 1) * D, h * D : (h + 1) * D] = wv

    in_maps = []
    for core in range(8):
        b, hg = core // 2, core % 2
        xs = np.ascontiguousarray(x[b, :, hg * P : (hg + 1) * P])
        in_maps.append(
            {
                "adjT": np.ascontiguousarray(adj[b].T),
                "xs": xs,
                "xsT": np.ascontiguousarray(xs.T),
                "w4": w4,
                "wblk": wblk,
                "eps1": eps1,
            }
        )
    return in_maps


def run(x, adj, W_qkv, eps, trace=False):
    """Run on 8 NeuronCores; returns (out, attn_weight, BassKernelResults)."""
    from concourse import bass_utils

    if "nc" not in _CACHE:
        _CACHE["nc"] = _build()
    nc = _CACHE["nc"]

    in_maps = _prep_inputs(
        np.asarray(x, np.float32), np.asarray(adj, np.float32),
        np.asarray(W_qkv, np.float32), np.asarray(eps, np.float32),
    )
    res = bass_utils.run_bass_kernel_spmd(
        nc, in_maps, core_ids=list(range(8)), trace=trace
    )

    out = np.empty((B, T, DIM), np.float32)
    attn = np.empty((B, HEADS, T, T), np.float32)
    for core in range(8):
        b, hg = core // 2, core % 2
        r = res.results[core]
        attn[b, hg * NH : (hg + 1) * NH] = r["attn_o"]
        out[b, :, hg * P : (hg + 1) * P] = r["out_o"]
    return out, attn, res


def kernel(x, adj, rep_adj_dis, W_qkv, eps):
    out, attn, _ = run(x, adj, W_qkv, eps, trace=False)
    return out, attn
